# revision 25
# baseline (speedup 1.0000x reference)
"""Trainium2 Bass kernel for the ATriplet loss (n=4096, d=512, 8 cores).

Math (per reference.py):
  dist[i,j] = sqrt(|xi|^2+|xj|^2-2 xi.xj)  (diagonal excluded)
  pos = 7 same-class dists per row, neg = 4088 other-class dists per row
  pos_logit = sum exp(40(1-pos)); neg_logit = sum exp(40(1-neg))
  a_lr = neg_logit/(pos_logit+neg_logit)
  trip[j,k] = log1p(exp(4(pos_k - neg_j))); valid = trip > 0.65
  loss_row = a_lr * sum(valid trip)/max(cnt,1);  loss = sum(loss_row)/sum(cnt)

Device strategy (row-parallel over 8 cores, 512 rows each):
  * Host rotates the (d-major) embedding per core so its own rows are
    columns 0..511 -> one SPMD program for all cores.
  * Split-precision matmul: x = hi + lo (bf16 pair); the Gram slab uses
    hi@hi + hi@lo + lo@hi (lo@lo ~ 2^-16 dropped). Contraction augmented
    with a K=2 row of [sq_hi; sq_lo] so PSUM = -2S + sq_col directly.
  * dist = exp(0.5*ln(d2 + sq_row)) -- the ln/exp pair keeps the whole
    kernel in ONE ACT table set (no sqrt-table thrash). Self-diagonal
    patched with +1e9 in PSUM so exp() underflow kills self terms.
  * a_lr from B^10 = exp(-40 d) built by DVE bf16 squarings of
    B = exp(-4 d) (ratio a_lr is scale-invariant).
  * Triplet phase uses exp(4(p-n)) = A_k*B_j separability; 7 compacted
    pos slots; per k: DVE tensor_scalar max(A_k*B, q) (bf16 4x),
    ACT Ln(1+t) with accum_out (paired to [128,8192]), DVE is_gt with
    accum_out for counts. Identity: sum_valid trip = sum ln(1+max(AB,q))
    - CINV*(PAIRS-cnt) where CINV = ln(1+bf16(q)) (patched/invalid pairs
    contribute exactly bf16(q)).
"""

import os
import sys

import numpy as np

if os.path.isdir("/opt/trn_rl_repo"):
    sys.path.insert(0, "/opt/trn_rl_repo")

import concourse.bass as bass
import concourse.tile as tile
from concourse import bacc, mybir
from concourse.bass_utils import run_bass_kernel_spmd

ALPHA = 40.0
BETA = 4.0
M_INST = 8          # samples per class
N_CORES = 8
F32 = mybir.dt.float32
BF16 = mybir.dt.bfloat16
ALU = mybir.AluOpType
AFT = mybir.ActivationFunctionType

# threshold constants: valid <=> exp(beta*(p-n)) > Q ; tk is bf16 so the
# invalid/patched slots contribute exactly ln(1 + bf16(Q)) each.
import ml_dtypes  # noqa: E402

Q = float(np.float32(np.expm1(np.float64(0.65))))
QBF = float(np.float64(ml_dtypes.bfloat16(Q)))
CINV = float(np.float32(np.log1p(np.float64(QBF))))
BIG = 1.0e9


def build_program(n=4096, rpc=512):
    """Build the single-core SPMD program. rpc = rows per core."""
    d = 512
    P = 128
    NT = rpc // P                # row tiles per core
    CW = 512                     # matmul chunk width (1 PSUM bank)
    NCW = n // CW
    KD = d // P                  # contraction tiles
    KP = 7                       # compacted pos slots
    PAIRS = float(KP * n)        # (j,k) grid per row incl. patched cols

    nc = bacc.Bacc("TRN2", target_bir_lowering=False, debug=False,
                   num_devices=N_CORES)

    # register extra bias consts (framework pre-registers only 0.0/1.0)
    for cname, cval in (("c40", ALPHA), ("cnq", -Q)):
        tcst = nc.alloc_sbuf_tensor(f"const-float32-{cname}", [128, 1], F32)
        nc.gpsimd.memset(tcst.ap(), cval)
        nc.const_aps.aps[(F32, cval)] = tcst.ap()
    nc.all_engine_barrier()

    xhi_d = nc.dram_tensor("xhi", [d, n], BF16, kind="ExternalInput")
    xlo_d = nc.dram_tensor("xlo", [d, n], BF16, kind="ExternalInput")
    bigi_d = nc.dram_tensor("bigi", [P, P], F32, kind="ExternalInput")
    g8_d = nc.dram_tensor("g8", [P, P], BF16, kind="ExternalInput")
    invg8_d = nc.dram_tensor("invg8", [P, P], BF16, kind="ExternalInput")
    rowm_d = nc.dram_tensor("rowm", [P, M_INST], F32, kind="ExternalInput")
    onescol_d = nc.dram_tensor("onescol", [P, 1], F32, kind="ExternalInput")
    onescolb_d = nc.dram_tensor("onescolb", [P, 1], BF16, kind="ExternalInput")
    ones2_d = nc.dram_tensor("ones2", [2, P], BF16, kind="ExternalInput")
    out_d = nc.dram_tensor("out", [1, 2], F32, kind="ExternalOutput")
    sqscr_d = nc.dram_tensor("sqscratch", [n], F32)  # internal scratch

    from concourse.bass import _add_dep_helper

    act_chain = [None]

    def chain(inst):
        if act_chain[0] is not None:
            _add_dep_helper(inst.ins, act_chain[0].ins, sync=False,
                            reason="act-order")
        act_chain[0] = inst
        return inst

    with tile.TileContext(nc) as tc:
        from contextlib import ExitStack
        with ExitStack() as ctx:
            cpool = ctx.enter_context(tc.tile_pool(name="consts", bufs=1))
            dpool = ctx.enter_context(tc.tile_pool(name="dist", bufs=2))
            hpool = ctx.enter_context(tc.tile_pool(name="hilo", bufs=1))
            spool = ctx.enter_context(tc.tile_pool(name="smalls", bufs=1))

            bigi = cpool.tile([P, P], F32, tag="bigi")
            g8 = cpool.tile([P, P], BF16, tag="g8")
            invg8 = cpool.tile([P, P], BF16, tag="invg8")
            rowm = cpool.tile([P, M_INST], F32, tag="rowm")
            onescol = cpool.tile([P, 1], F32, tag="onescol")
            onescolb = cpool.tile([P, 1], BF16, tag="onescolb")
            ones2 = cpool.tile([2, P], BF16, tag="ones2")
            nc.sync.dma_start(bigi[:], bigi_d[:])
            nc.sync.dma_start(g8[:], g8_d[:])
            nc.sync.dma_start(invg8[:], invg8_d[:])
            nc.sync.dma_start(rowm[:], rowm_d[:])
            nc.sync.dma_start(onescol[:], onescol_d[:])
            nc.sync.dma_start(onescolb[:], onescolb_d[:])
            nc.sync.dma_start(ones2[:], ones2_d[:])

            hi = [hpool.tile([P, n], BF16, tag=f"hi{k}", name=f"hi{k}")
                  for k in range(KD)]
            lo = [hpool.tile([P, n], BF16, tag=f"lo{k}", name=f"lo{k}")
                  for k in range(KD)]
            whi = hpool.tile([P, KD, rpc], BF16, tag="whi")
            sq2 = hpool.tile([2, n], BF16, tag="sq2")

            sqrow = spool.tile([P, NT], F32, tag="sqrow")
            pos8 = spool.tile([P, NT, M_INST], F32, tag="pos8")
            loss_all = spool.tile([P, NT], F32, tag="loss_all")
            cnt_all = spool.tile([P, NT], F32, tag="cnt_all")

            # ------------- setup: load hi/lo, column norms -------------
            with ExitStack() as p1:
                xsqp = p1.enter_context(tc.tile_pool(name="xsq", bufs=2))
                qpool = p1.enter_context(tc.tile_pool(name="sqs", bufs=1))
                psq_p = p1.enter_context(
                    tc.tile_pool(name="psq", bufs=1,
                                 space=bass.MemorySpace.PSUM))

                sq_sb = qpool.tile([1, n], F32, tag="sqsb")

                xhi_r = xhi_d.ap().rearrange("(kd p) c -> kd p c", p=P)
                xlo_r = xlo_d.ap().rearrange("(kd p) c -> kd p c", p=P)
                for kd in range(KD):
                    nc.sync.dma_start(hi[kd][:], xhi_r[kd])
                    nc.vector.tensor_scalar(
                        out=whi[:, kd, :], in0=hi[kd][:, 0:rpc],
                        scalar1=-2.0, scalar2=None, op0=ALU.mult)
                for kd in range(KD):
                    nc.sync.dma_start(lo[kd][:], xlo_r[kd])

                # column norms via ones-matmul of squared entries
                for ch in range(n // 512):
                    psq = psq_p.tile([1, 512], F32, tag="psq")
                    for kd in range(KD):
                        xsq = xsqp.tile([P, 512], BF16, tag="xsq")
                        sl = hi[kd][:, 512 * ch:512 * (ch + 1)]
                        nc.vector.tensor_tensor(out=xsq[:], in0=sl, in1=sl,
                                                op=ALU.mult)
                        nc.tensor.matmul(psq[:], onescolb[:], xsq[:],
                                         start=(kd == 0), stop=(kd == KD - 1))
                    chain(nc.scalar.copy(sq_sb[:, 512 * ch:512 * (ch + 1)], psq[:]))

                # split sq into bf16 hi/lo pair rows (row 1 via DMA --
                # compute engines cannot start at partition 1); chunked:
                # single-partition ops are FD-serial, keep them small
                sqlo_t = qpool.tile([1, n], BF16, tag="sqlo")
                for ch in range(n // 512):
                    slq = slice(512 * ch, 512 * (ch + 1))
                    chain(nc.scalar.copy(sq2[0:1, slq], sq_sb[0:1, slq]))
                    nc.vector.scalar_tensor_tensor(
                        out=sqlo_t[0:1, slq], in0=sq_sb[0:1, slq], scalar=0.0,
                        in1=sq2[0:1, slq], op0=ALU.bypass, op1=ALU.subtract)
                nc.sync.dma_start(sq2[1:2, :], sqlo_t[:])

                # f32 row norms per tile: bounce through DRAM
                nc.sync.dma_start(
                    sqscr_d.ap()[0:rpc].rearrange("(a b) -> a b", a=1),
                    sq_sb[0:1, 0:rpc])
                sq_t = sqscr_d.ap().rearrange("(t p) -> t p", p=P)
                for t in range(NT):
                    nc.sync.dma_start(
                        sqrow[:, t:t + 1],
                        sq_t[t].rearrange("(p o) -> p o", o=1))

            # ------------- per-tile: distances + logits + triplets -------
            with ExitStack() as p2:
                s_p = p2.enter_context(
                    tc.tile_pool(name="spsum", bufs=2,
                                 space=bass.MemorySpace.PSUM))
                lnt_p = p2.enter_context(tc.tile_pool(name="lntmp", bufs=1))
                dd_p = p2.enter_context(tc.tile_pool(name="ddiag", bufs=2))
                bpool = p2.enter_context(tc.tile_pool(name="bbuf", bufs=2))
                tkp = p2.enter_context(tc.tile_pool(name="tk", bufs=2))
                scrap = p2.enter_context(tc.tile_pool(name="scrap", bufs=2))
                sm = p2.enter_context(tc.tile_pool(name="sm2", bufs=2))

                pairs = [(0, 1), (2, 3), (4, 5), (6,)]

                for t in range(NT):
                    # ---- matmul slab: psum = -2S + sq_col ----
                    # two psum halves; each matmul writes one bank
                    HB = NCW // 2
                    psums = [s_p.tile([P, HB * CW], F32, tag="spsum",
                                      name=f"ps{t}_{c}") for c in range(2)]
                    first = True
                    for (w, r) in ((whi, hi), (whi, lo)):
                        for kd in range(KD):
                            for c in range(NCW):
                                nc.tensor.matmul(
                                    psums[c // HB][:, CW * (c % HB):
                                                   CW * (c % HB + 1)],
                                    w[:, kd, P * t:P * (t + 1)],
                                    r[kd][:, CW * c:CW * (c + 1)],
                                    start=first, stop=False)
                            first = False
                    for c in range(NCW):
                        nc.tensor.matmul(
                            psums[c // HB][:, CW * (c % HB):
                                           CW * (c % HB + 1)],
                            ones2[:], sq2[:, CW * c:CW * (c + 1)],
                            start=False, stop=True)
                    # self-diagonal -> +1e9 (kills self terms downstream)
                    blk = psums[0][:, P * t:P * t + P]
                    nc.vector.tensor_tensor(out=blk, in0=blk, in1=bigi[:],
                                            op=ALU.add)

                    # ---- dist = exp(0.5 ln(d2 + sq_row)) (bf16 tile) ----
                    # all LNs first, then all EXPs: walrus puts ln and exp
                    # in different ACT table sets; interleaving would load
                    # tables on every transition (~1.5us each)
                    dist = dpool.tile([P, n], BF16, tag="dist",
                                      name=f"dist{t}")
                    lntmp = lnt_p.tile([P, n], BF16, tag="lntmp")
                    dd_ln = dd_p.tile([P, P], F32, tag="ddln")
                    ddiag = dd_p.tile([P, P], F32, tag="ddiag")
                    for c in range(2):
                        chain(nc.scalar.activation(
                            out=lntmp[:, HB * CW * c:HB * CW * (c + 1)],
                            in_=psums[c][:], func=AFT.Ln,
                            bias=sqrow[:, t:t + 1], scale=1.0))
                    chain(nc.scalar.activation(
                        out=dd_ln[:], in_=psums[0][:, P * t:P * t + P],
                        func=AFT.Ln, bias=sqrow[:, t:t + 1], scale=1.0))
                    chain(nc.scalar.activation(
                        out=dist[:], in_=lntmp[:], func=AFT.Exp, bias=0.0,
                        scale=0.5))
                    chain(nc.scalar.activation(
                        out=ddiag[:], in_=dd_ln[:], func=AFT.Exp,
                        bias=0.0, scale=0.5))

                    # ---- pos slots: gather own-group, compact 8 -> 7 ----
                    for g in range(P // M_INST):
                        r0 = M_INST * g
                        nc.sync.dma_start(
                            pos8[r0:r0 + M_INST, t, :],
                            ddiag[r0:r0 + M_INST, r0:r0 + M_INST])
                    p7tmp = sm.tile([P, KP, M_INST], F32, tag="p7tmp")
                    for s in range(M_INST):
                        if s > 0:
                            nc.vector.tensor_scalar(
                                out=p7tmp[:, 0:s, s:s + 1],
                                in0=pos8[:, t, 0:s].rearrange(
                                    "p (a o) -> p a o", o=1),
                                scalar1=rowm[:, s:s + 1], scalar2=None,
                                op0=ALU.mult)
                        if s < KP:
                            nc.vector.tensor_scalar(
                                out=p7tmp[:, s:KP, s:s + 1],
                                in0=pos8[:, t, s + 1:M_INST].rearrange(
                                    "p (a o) -> p a o", o=1),
                                scalar1=rowm[:, s:s + 1], scalar2=None,
                                op0=ALU.mult)
                    pos7 = sm.tile([P, KP], F32, tag="pos7")
                    nc.vector.reduce_sum(pos7[:], p7tmp[:],
                                         axis=mybir.AxisListType.X)
                    a7 = sm.tile([P, KP], F32, tag="a7")
                    chain(nc.scalar.activation(out=a7[:], in_=pos7[:],
                                               func=AFT.Exp, bias=0.0,
                                               scale=BETA))

                    # ---- B = exp(-4 dist); logits P = exp(40(1-d)) ----
                    b_t = bpool.tile([P, n], BF16, tag="bbuf")
                    chain(nc.scalar.activation(out=b_t[:], in_=dist[:],
                                                func=AFT.Exp, bias=0.0,
                                                scale=-BETA))
                    p_t = scrap.tile([P, 2 * n], BF16, tag="scrap",
                                     name=f"pt{t}")
                    total = sm.tile([P, 1], F32, tag="total")
                    chain(nc.scalar.activation(out=p_t[:, 0:n], in_=dist[:],
                                                func=AFT.Exp, bias=ALPHA,
                                                scale=-ALPHA,
                                                accum_out=total[:]))
                    posl = sm.tile([P, 1], F32, tag="posl")
                    s128 = sm.tile([P, P], BF16, tag="s128")
                    nc.vector.scalar_tensor_tensor(
                        out=s128[:], in0=p_t[:, P * t:P * t + P],
                        scalar=0.0, in1=g8[:], op0=ALU.bypass, op1=ALU.mult,
                        accum_out=posl[:])
                    rtot = sm.tile([P, 1], F32, tag="rtot")
                    nc.vector.reciprocal(rtot[:], total[:])
                    ratio = sm.tile([P, 1], F32, tag="ratio")
                    nc.vector.tensor_tensor(out=ratio[:], in0=posl[:],
                                            in1=rtot[:], op=ALU.mult)
                    alr = sm.tile([P, 1], F32, tag="alr")
                    nc.vector.tensor_scalar(out=alr[:], in0=ratio[:],
                                            scalar1=-1.0, scalar2=1.0,
                                            op0=ALU.mult, op1=ALU.add)
                    bblk = b_t[:, P * t:P * t + P]
                    nc.vector.tensor_tensor(out=bblk, in0=bblk, in1=invg8[:],
                                            op=ALU.mult)

                    # ---- triplet k-loop (paired) ----
                    lnacc = sm.tile([P, len(pairs)], F32, tag="lnacc")
                    cntacc = sm.tile([P, len(pairs)], F32, tag="cntacc")
                    for pi, pr in enumerate(pairs):
                        tk = tkp.tile([P, 2 * n], BF16, tag="tk")
                        for j, k in enumerate(pr):
                            nc.vector.tensor_scalar(
                                out=tk[:, n * j:n * (j + 1)], in0=b_t[:],
                                scalar1=a7[:, k:k + 1], scalar2=Q,
                                op0=ALU.mult, op1=ALU.max)
                        pw = n * len(pr)
                        msk = scrap.tile([P, 2 * n], BF16, tag="scrap")
                        nc.vector.tensor_scalar(
                            out=msk[:, 0:pw], in0=tk[:, 0:pw],
                            scalar1=Q, scalar2=None, op0=ALU.is_gt,
                            op1=ALU.add, accum_out=cntacc[:, pi:pi + 1])
                        lns = scrap.tile([P, 2 * n], BF16, tag="scrap")
                        chain(nc.scalar.activation(
                            out=lns[:, 0:pw], in_=tk[:, 0:pw], func=AFT.Ln,
                            bias=1.0, scale=1.0,
                            accum_out=lnacc[:, pi:pi + 1]))

                    lnrow = sm.tile([P, 1], F32, tag="lnrow")
                    cntrow = sm.tile([P, 1], F32, tag="cntrow")
                    nc.vector.reduce_sum(lnrow[:], lnacc[:],
                                         axis=mybir.AxisListType.X)
                    nc.vector.reduce_sum(cntrow[:], cntacc[:],
                                         axis=mybir.AxisListType.X)
                    # loss_row = alr*(lnrow + CINV*cnt - CINV*PAIRS)/max(cnt,1)
                    tmp1 = sm.tile([P, 1], F32, tag="tmp1")
                    nc.vector.scalar_tensor_tensor(
                        out=tmp1[:], in0=cntrow[:], scalar=CINV,
                        in1=lnrow[:], op0=ALU.mult, op1=ALU.add)
                    dn = sm.tile([P, 1], F32, tag="dn")
                    nc.vector.tensor_scalar(out=dn[:], in0=cntrow[:],
                                            scalar1=1.0, scalar2=None,
                                            op0=ALU.max)
                    num = sm.tile([P, 1], F32, tag="num")
                    nc.vector.tensor_scalar(out=num[:], in0=tmp1[:],
                                            scalar1=-CINV * PAIRS,
                                            scalar2=None, op0=ALU.add)
                    rdn = sm.tile([P, 1], F32, tag="rdn")
                    nc.vector.reciprocal(rdn[:], dn[:])
                    rr = sm.tile([P, 1], F32, tag="rr")
                    nc.vector.tensor_tensor(out=rr[:], in0=num[:],
                                            in1=rdn[:], op=ALU.mult)
                    nc.vector.tensor_tensor(out=loss_all[:, t:t + 1],
                                            in0=rr[:], in1=alr[:],
                                            op=ALU.mult)
                    nc.vector.tensor_copy(cnt_all[:, t:t + 1], cntrow[:])

            # ---- final reduction to 2 scalars (PSUM freed by p2 exit) ----
            fin2 = spool.tile([P, 2], F32, tag="fin2")
            nc.vector.reduce_sum(fin2[:, 0:1], loss_all[:],
                                 axis=mybir.AxisListType.X)
            nc.vector.reduce_sum(fin2[:, 1:2], cnt_all[:],
                                 axis=mybir.AxisListType.X)
            osb = spool.tile([1, 2], F32, tag="osb")
            with tc.tile_pool(name="pfin", bufs=1,
                              space=bass.MemorySpace.PSUM) as pf:
                pfin = pf.tile([1, 2], F32, tag="pfin")
                nc.tensor.matmul(pfin[:], onescol[:], fin2[:],
                                 start=True, stop=True)
                chain(nc.scalar.copy(osb[:], pfin[:]))
                nc.sync.dma_start(out_d[:], osb[:])
    nc.compile()
    return nc


def make_consts(P=128):
    g8 = np.kron(np.eye(P // M_INST, dtype=np.float32),
                 np.ones((M_INST, M_INST), dtype=np.float32))
    rowm = np.zeros((P, M_INST), dtype=np.float32)
    rowm[np.arange(P), np.arange(P) % M_INST] = 1.0
    consts = {
        "bigi": (BIG * np.eye(P)).astype(np.float32),
        "g8": g8.astype(ml_dtypes.bfloat16),
        "invg8": (1.0 - g8).astype(ml_dtypes.bfloat16),
        "rowm": rowm,
        "onescol": np.ones((P, 1), dtype=np.float32),
        "onescolb": np.ones((P, 1), dtype=ml_dtypes.bfloat16),
        "ones2": np.ones((2, P), dtype=ml_dtypes.bfloat16),
    }
    return consts


def make_in_maps(X, n_cores=N_CORES):
    n, d = X.shape
    rpc = n // n_cores
    XT = np.ascontiguousarray(X.T.astype(np.float32))
    XHI = XT.astype(ml_dtypes.bfloat16)
    XLO = (XT - XHI.astype(np.float32)).astype(ml_dtypes.bfloat16)
    consts = make_consts()
    in_maps = []
    for c in range(n_cores):
        m = {"xhi": np.ascontiguousarray(np.roll(XHI, -rpc * c, axis=1)),
             "xlo": np.ascontiguousarray(np.roll(XLO, -rpc * c, axis=1))}
        m.update(consts)
        in_maps.append(m)
    return in_maps


def combine(results):
    ls = 0.0
    cs = 0.0
    for r in results:
        o = np.asarray(r["out"], dtype=np.float64).reshape(-1)
        ls += o[0]
        cs += o[1]
    if cs <= 0:
        return np.float32(0.0)
    return np.float32(ls / cs)


def kernel(inputs, targets=None, _trace=False, _tmpdir=None):
    X = np.asarray(inputs, dtype=np.float32)
    n, d = X.shape
    nc = build_program(n=n, rpc=n // N_CORES)
    in_maps = make_in_maps(X)
    res = run_bass_kernel_spmd(nc, in_maps, list(range(N_CORES)),
                               trace=_trace, tmpdir=_tmpdir)
    out = combine(res.results)
    if _trace:
        return out, res
    return out


if __name__ == "__main__":
    rng = np.random.default_rng(0)
    X = (0.03 * rng.standard_normal((4096, 512))).astype(np.float32)
    print(kernel(X))
